# revision 28
# baseline (speedup 1.0000x reference)
"""Cosine-similarity loss kernel for Trainium2 (8 NeuronCores, SPMD).

Computes mean(offdiag(xn @ yn^T)) for row-normalized x, y of shape
(N, D) = (4096, 512), via the identity

    mean = [ (Sx . Sy) - sum_n (xn_n . yn_n) ] / N^2
    Sx = sum_n xn_n,  Sy = sum_m yn_m

so the N x N similarity matrix is never formed. Rows are sharded across
the 8 cores (512 rows each, 4 partition-tiles of 128). Per core:

  POOL: p_t = x * y elementwise (runs right after the DMAs, off the
        critical path)
  ACT : row sums-of-squares (Square activation with accum), sqrt
        (Square/Sqrt share one activation table: sqrt_and_friends)
  DVE : 1/norm, xnacc += rnx*x, ynacc += rny*y,
        dtmp_t = sum_d rnx*p_t (single-stream tensor_scalar, 2x mode)
  out : xnacc via the ACT HWDGE ring as soon as it is final;
        [ynacc | dtmp | rn] via the SP ring. Host sums partitions/cores,
        applies the rny factor to dtmp, and takes the 512-length dot.

Raw bass with explicit semaphores; every instruction carries at most one
sync-wait (this walrus rejects multi-wait instructions); cross-engine
ordering is established transitively through semaphore chains.
"""

import numpy as np

N = 4096
D = 512
NCORES = 8
ROWS = N // NCORES  # 512 rows per core
P = 128
NTILES = ROWS // P  # 4
# ACC columns: [xnacc 0:512 | ynacc 512:1024 | dtmp 1024:1028 | rn 1028:1036]
DT0 = 2 * D
RN0 = 2 * D + NTILES
OUTW = RN0 + 2 * NTILES

_CACHE = {}

# tiles whose ssy (row sum-of-squares of y) runs on DVE instead of ACT --
# balances the two engines' per-tile load (tuned via cost-model sweep)
SSY_ON_DVE = (0,)
FINAL_WAIT = True
# tiles whose diag reduction runs on ACT (Copy with scale AP + accum) --
# ACT is idle after the last sqrt, so this takes the diag off the DVE tail
DG_ON_ACT = (3,)
# input DMA issue order (tuned via cost-model sweep)
DMA_ORDER = (("x", 0), ("y", 0), ("x", 1), ("y", 1),
             ("x", 2), ("y", 2), ("x", 3), ("y", 3))


def _build_program(ssy_on_dve=None, final_wait=None, dma_order=None,
                   dg_on_act=None):
    import concourse.bass as bass
    from concourse import mybir
    from contextlib import ExitStack

    if ssy_on_dve is None:
        ssy_on_dve = SSY_ON_DVE
    if final_wait is None:
        final_wait = FINAL_WAIT
    if dma_order is None:
        dma_order = DMA_ORDER
    if dg_on_act is None:
        dg_on_act = DG_ON_ACT

    f32 = mybir.dt.float32
    mult = mybir.AluOpType.mult
    add = mybir.AluOpType.add
    bypass = mybir.AluOpType.bypass
    Square = mybir.ActivationFunctionType.Square
    Sqrt = mybir.ActivationFunctionType.Sqrt

    nc = bass.Bass("TRN2", target_bir_lowering=False)

    x_d = nc.dram_tensor("x", [ROWS, D], f32, kind="ExternalInput").ap()
    y_d = nc.dram_tensor("y", [ROWS, D], f32, kind="ExternalInput").ap()
    o_d = nc.dram_tensor("o", [P, OUTW], f32, kind="ExternalOutput").ap()

    with (
        ExitStack() as _sems,
        nc.semaphore("sv") as sv,    # DVE progress
        nc.semaphore("sa") as sa,    # ACT progress
        nc.semaphore("sg") as sg,    # POOL progress
        nc.semaphore("sz") as sz,    # zero-bias tile ready
        nc.semaphore("do_") as do_,  # output DMA completion
        nc.sbuf_tensor("zb", [P, 1], f32) as zb_h,
        nc.sbuf_tensor("X", [P, NTILES, D], f32) as X_h,
        nc.sbuf_tensor("Y", [P, NTILES, D], f32) as Y_h,
        nc.sbuf_tensor("PT", [P, NTILES, D], f32) as PT_h,
        nc.sbuf_tensor("ss", [P, NTILES, 2], f32) as ss_h,
        nc.sbuf_tensor("nrm", [P, NTILES, 2], f32) as nrm_h,
        nc.sbuf_tensor("scrv", [P, D], f32) as scrv_h,
        nc.sbuf_tensor("scra", [P, 2, D], f32) as scra_h,
        nc.sbuf_tensor("ACC", [P, OUTW], f32) as acc_h,
    ):
        X = X_h.ap()
        Y = Y_h.ap()
        PT = PT_h.ap()
        zb = zb_h.ap()
        ss = ss_h.ap()
        nrm = nrm_h.ap()
        scrv = scrv_h.ap()
        scra = scra_h.ap()
        ACC = acc_h.ap()
        xnacc = ACC[:, 0:D]
        ynacc = ACC[:, D:2 * D]

        # explicit zero bias for activations (avoids the implicit const-AP
        # path; framework preamble consts exist anyway, but zb keeps the
        # activation biases off the ACT critical path)
        nc.gpsimd.memset(zb, 0.0).then_inc(sz, 1)

        # ---- input DMAs: dedicated completion semaphore per DMA so every
        # wait is an unambiguous milestone (HWDGE completions can reorder).
        dx = [_sems.enter_context(nc.semaphore(f"dx{t}")) for t in range(NTILES)]
        dy = [_sems.enter_context(nc.semaphore(f"dy{t}")) for t in range(NTILES)]
        for kind, t in dma_order:
            if kind == "x":
                nc.sync.dma_start(
                    out=X[:, t, :], in_=x_d[t * P:(t + 1) * P, :]
                ).then_inc(dx[t], 16)
            else:
                nc.sync.dma_start(
                    out=Y[:, t, :], in_=y_d[t * P:(t + 1) * P, :]
                ).then_inc(dy[t], 16)

        nc.scalar.wait_ge(sz, 1)  # bias tile ready before first activation

        sa_t = 0  # ACT tick
        sv_t = 0  # DVE tick
        sg_t = 0  # POOL tick
        sqrt_tick = [0] * NTILES
        recip_tick = [0] * NTILES
        xnacc_tick = [0] * NTILES
        p_tick = [0] * NTILES

        for t in range(NTILES):
            Xt = X[:, t, :]
            Yt = Y[:, t, :]
            Pt = PT[:, t, :]
            rn_t = ACC[:, RN0 + 2 * t:RN0 + 2 * t + 2]
            rnx = ACC[:, RN0 + 2 * t:RN0 + 2 * t + 1]
            rny = ACC[:, RN0 + 2 * t + 1:RN0 + 2 * t + 2]
            dt_ = ACC[:, DT0 + t:DT0 + t + 1]

            # ---- POOL: p_t = x * y (independent of rn; runs early)
            nc.gpsimd.wait_ge(dx[t], 16)
            nc.gpsimd.wait_ge(dy[t], 16)
            nc.gpsimd.tensor_mul(Pt, Xt, Yt).then_inc(sg, 1)
            sg_t += 1
            p_tick[t] = sg_t

            # ---- ACT: ssx via Square-with-accum
            nc.scalar.wait_ge(dx[t], 16)
            nc.scalar.activation(
                out=scra[:, 0, :], in_=Xt, func=Square, bias=zb,
                accum_out=ss[:, t, 0:1],
            ).then_inc(sa, 1)
            sa_t += 1
            sqx_tick = sa_t

            if t in ssy_on_dve:
                # ---- DVE: ssy_t -- balances ACT/DVE load; also pulls the
                # tile-0 sqrt earlier than the serial ACT chain would allow
                nc.vector.wait_ge(dy[t], 16)
                nc.vector.scalar_tensor_tensor(
                    out=scrv, in0=Yt, scalar=1.0, in1=Yt,
                    op0=bypass, op1=mult, accum_out=ss[:, t, 1:2],
                ).then_inc(sv, 1)
                sv_t += 1
                ssy_sv_tick = sv_t
                # sqrt waits own sqx accum + DVE ssy
                nc.scalar.wait_ge(sa, sqx_tick)
                nc.scalar.wait_ge(sv, ssy_sv_tick)
            else:
                nc.scalar.wait_ge(dy[t], 16)
                nc.scalar.activation(
                    out=scra[:, 1, :], in_=Yt, func=Square, bias=zb,
                    accum_out=ss[:, t, 1:2],
                ).then_inc(sa, 1)
                sa_t += 1
                nc.scalar.wait_ge(sa, sa_t)  # own accum-out hazard

            # ---- ACT: nrm = sqrt(ss)
            nc.scalar.activation(
                out=nrm[:, t, :], in_=ss[:, t, :], func=Sqrt, bias=zb,
            ).then_inc(sa, 1)
            sa_t += 1
            sqrt_tick[t] = sa_t

            # ---- DVE: rn = 1/nrm
            nc.vector.wait_ge(sa, sqrt_tick[t])
            nc.vector.reciprocal(out=rn_t, in_=nrm[:, t, :]).then_inc(sv, 1)
            sv_t += 1
            recip_tick[t] = sv_t

            # ---- DVE: xnacc += rnx * x   (scalar-ptr setup hazard on rn)
            nc.vector.wait_ge(sv, recip_tick[t])
            if t == 0:
                nc.vector.tensor_scalar_mul(xnacc, Xt, rnx).then_inc(sv, 1)
            else:
                nc.vector.scalar_tensor_tensor(
                    out=xnacc, in0=Xt, scalar=rnx, in1=xnacc,
                    op0=mult, op1=add,
                ).then_inc(sv, 1)
            sv_t += 1
            xnacc_tick[t] = sv_t

            # ---- DVE: ynacc += rny * y
            if t == 0:
                nc.vector.tensor_scalar_mul(ynacc, Yt, rny).then_inc(sv, 1)
            else:
                nc.vector.scalar_tensor_tensor(
                    out=ynacc, in0=Yt, scalar=rny, in1=ynacc,
                    op0=mult, op1=add,
                ).then_inc(sv, 1)
            sv_t += 1

            # ---- dtmp_t = sum_d rnx * p_t
            if t in dg_on_act:
                # ACT is idle after its last sqrt: Copy-with-scale + accum
                # takes the diag off the DVE tail for the final tile(s)
                nc.scalar.wait_ge(sg, p_tick[t])
                nc.scalar.wait_ge(sv, recip_tick[t])
                nc.scalar.activation(
                    out=scra[:, 0, :], in_=Pt,
                    func=mybir.ActivationFunctionType.Copy,
                    scale=rnx, accum_out=dt_,
                ).then_inc(sa, 1)
                sa_t += 1
            else:
                # DVE single-stream tensor_scalar (2x mode)
                nc.vector.wait_ge(sg, p_tick[t])
                nc.vector.tensor_scalar(
                    out=scrv, in0=Pt, scalar1=rnx, scalar2=0.0,
                    op0=mult, op1=add, accum_out=dt_,
                ).then_inc(sv, 1)
                sv_t += 1

        # ---- tail: xnacc out on the ACT HWDGE ring as soon as it's final,
        # the rest on the SP ring once the last DVE op lands.
        nc.scalar.wait_ge(sv, xnacc_tick[NTILES - 1])
        nc.scalar.dma_start(out=o_d[:, 0:D], in_=xnacc).then_inc(do_, 16)
        nc.sync.wait_ge(sv, sv_t)
        if dg_on_act:
            nc.sync.wait_ge(sa, sa_t)
        nc.sync.dma_start(out=o_d[:, D:OUTW], in_=ACC[:, D:OUTW]).then_inc(do_, 16)
        if final_wait:
            nc.sync.wait_ge(do_, 32)

    return nc


def _get_program():
    if "nc" not in _CACHE:
        _CACHE["nc"] = _build_program()
    return _CACHE["nc"]


def kernel(x: np.ndarray, y: np.ndarray) -> np.ndarray:
    from concourse import bass_utils

    nc = _get_program()

    xs = np.ascontiguousarray(np.asarray(x, dtype=np.float32).reshape(N, D))
    ys = np.ascontiguousarray(np.asarray(y, dtype=np.float32).reshape(N, D))

    in_maps = [
        {"x": xs[c * ROWS:(c + 1) * ROWS], "y": ys[c * ROWS:(c + 1) * ROWS]}
        for c in range(NCORES)
    ]
    res = bass_utils.run_bass_kernel_spmd(nc, in_maps, core_ids=list(range(NCORES)))

    sx = np.zeros(D, dtype=np.float64)
    sy = np.zeros(D, dtype=np.float64)
    dg = 0.0
    for r in res.results:
        o = r["o"].astype(np.float64)
        sx += o[:, 0:D].sum(axis=0)
        sy += o[:, D:2 * D].sum(axis=0)
        dtmp = o[:, DT0:DT0 + NTILES]                    # rnx * (x . y)
        rny = o[:, RN0 + 1:RN0 + 2 * NTILES:2]           # (P, NTILES)
        dg += (dtmp * rny).sum()

    val = (np.dot(sx, sy) - dg) / float(N * N)
    return np.array(val, dtype=np.float32)


# revision 32
# speedup vs baseline: 1.0152x; 1.0152x over previous
"""Cosine-similarity loss kernel for Trainium2 (8 NeuronCores, SPMD).

Computes mean(offdiag(xn @ yn^T)) for row-normalized x, y of shape
(N, D) = (4096, 512), via the identity

    mean = [ (Sx . Sy) - sum_n (xn_n . yn_n) ] / N^2
    Sx = sum_n xn_n,  Sy = sum_m yn_m

so the N x N similarity matrix is never formed. Rows are sharded across
the 8 cores (512 rows each, 4 partition-tiles of 128). Per core:

  POOL: p_t = x * y elementwise (runs right after the DMAs, off the
        critical path)
  ACT : row sums-of-squares (Square activation with accum), sqrt
        (Square/Sqrt share one activation table: sqrt_and_friends)
  DVE : 1/norm, xnacc += rnx*x, ynacc += rny*y,
        dtmp_t = sum_d rnx*p_t (single-stream tensor_scalar, 2x mode)
  out : xnacc via the ACT HWDGE ring as soon as it is final;
        [ynacc | dtmp | rn] via the SP ring. Host sums partitions/cores,
        applies the rny factor to dtmp, and takes the 512-length dot.

Raw bass with explicit semaphores; every instruction carries at most one
sync-wait (this walrus rejects multi-wait instructions); cross-engine
ordering is established transitively through semaphore chains.
"""

import numpy as np

N = 4096
D = 512
NCORES = 8
ROWS = N // NCORES  # 512 rows per core
P = 128
NTILES = ROWS // P  # 4
# ACC columns: [xnacc 0:512 | ynacc 512:1024 | dtmp 1024:1028 | rn 1028:1036]
DT0 = 2 * D
RN0 = 2 * D + NTILES
OUTW = RN0 + 2 * NTILES

_CACHE = {}

# tiles whose ssy (row sum-of-squares of y) runs on DVE instead of ACT --
# balances the two engines' per-tile load (tuned via cost-model sweep)
SSY_ON_DVE = (0,)
FINAL_WAIT = True
# tiles whose diag reduction runs on ACT (Copy with scale AP + accum) --
# ACT is idle after the last sqrt, so this takes the diag off the DVE tail
DG_ON_ACT = (3,)
# split tile-0's sqrt/recip into separate x and y halves: xn0 only needs
# rnx, so it can start before ssy0 -> sqrt(y) resolves; ACT's bubble
# before the combined sqrt0 is filled with sqx1 instead
SPLIT_RN0 = True
# place tile-0's sqrt(y) after sqx1 in the ACT stream (True) or
# immediately after sqrt(x) (False)
SQRTY0_AFTER_SQX1 = True
# input DMA issue order (tuned via cost-model sweep)
DMA_ORDER = (("x", 0), ("y", 0), ("x", 1), ("y", 1),
             ("x", 2), ("y", 2), ("x", 3), ("y", 3))


def _build_program(ssy_on_dve=None, final_wait=None, dma_order=None,
                   dg_on_act=None, split_rn0=None, sqrty0_after_sqx1=None):
    import concourse.bass as bass
    from concourse import mybir
    from contextlib import ExitStack

    if ssy_on_dve is None:
        ssy_on_dve = SSY_ON_DVE
    if final_wait is None:
        final_wait = FINAL_WAIT
    if dma_order is None:
        dma_order = DMA_ORDER
    if dg_on_act is None:
        dg_on_act = DG_ON_ACT
    if split_rn0 is None:
        split_rn0 = SPLIT_RN0
    if sqrty0_after_sqx1 is None:
        sqrty0_after_sqx1 = SQRTY0_AFTER_SQX1

    f32 = mybir.dt.float32
    mult = mybir.AluOpType.mult
    add = mybir.AluOpType.add
    bypass = mybir.AluOpType.bypass
    Square = mybir.ActivationFunctionType.Square
    Sqrt = mybir.ActivationFunctionType.Sqrt

    nc = bass.Bass("TRN2", target_bir_lowering=False)

    x_d = nc.dram_tensor("x", [ROWS, D], f32, kind="ExternalInput").ap()
    y_d = nc.dram_tensor("y", [ROWS, D], f32, kind="ExternalInput").ap()
    o_d = nc.dram_tensor("o", [P, OUTW], f32, kind="ExternalOutput").ap()

    with (
        ExitStack() as _sems,
        nc.semaphore("sv") as sv,    # DVE progress
        nc.semaphore("sa") as sa,    # ACT progress
        nc.semaphore("sg") as sg,    # POOL progress
        nc.semaphore("sz") as sz,    # zero-bias tile ready
        nc.semaphore("do_") as do_,  # output DMA completion
        nc.sbuf_tensor("zb", [P, 1], f32) as zb_h,
        nc.sbuf_tensor("X", [P, NTILES, D], f32) as X_h,
        nc.sbuf_tensor("Y", [P, NTILES, D], f32) as Y_h,
        nc.sbuf_tensor("PT", [P, NTILES, D], f32) as PT_h,
        nc.sbuf_tensor("ss", [P, NTILES, 2], f32) as ss_h,
        nc.sbuf_tensor("nrm", [P, NTILES, 2], f32) as nrm_h,
        nc.sbuf_tensor("scrv", [P, D], f32) as scrv_h,
        nc.sbuf_tensor("scra", [P, 2, D], f32) as scra_h,
        nc.sbuf_tensor("ACC", [P, OUTW], f32) as acc_h,
    ):
        X = X_h.ap()
        Y = Y_h.ap()
        PT = PT_h.ap()
        zb = zb_h.ap()
        ss = ss_h.ap()
        nrm = nrm_h.ap()
        scrv = scrv_h.ap()
        scra = scra_h.ap()
        ACC = acc_h.ap()
        xnacc = ACC[:, 0:D]
        ynacc = ACC[:, D:2 * D]

        # explicit zero bias for activations (avoids the implicit const-AP
        # path; framework preamble consts exist anyway, but zb keeps the
        # activation biases off the ACT critical path)
        nc.gpsimd.memset(zb, 0.0).then_inc(sz, 1)

        # ---- input DMAs: dedicated completion semaphore per DMA so every
        # wait is an unambiguous milestone (HWDGE completions can reorder).
        dx = [_sems.enter_context(nc.semaphore(f"dx{t}")) for t in range(NTILES)]
        dy = [_sems.enter_context(nc.semaphore(f"dy{t}")) for t in range(NTILES)]
        for kind, t in dma_order:
            if kind == "x":
                nc.sync.dma_start(
                    out=X[:, t, :], in_=x_d[t * P:(t + 1) * P, :]
                ).then_inc(dx[t], 16)
            else:
                nc.sync.dma_start(
                    out=Y[:, t, :], in_=y_d[t * P:(t + 1) * P, :]
                ).then_inc(dy[t], 16)

        nc.scalar.wait_ge(sz, 1)  # bias tile ready before first activation

        sa_t = 0  # ACT tick
        sv_t = 0  # DVE tick
        sg_t = 0  # POOL tick
        sqrt_tick = [0] * NTILES
        recip_tick = [0] * NTILES
        xnacc_tick = [0] * NTILES
        p_tick = [0] * NTILES

        split_rn0 = split_rn0 and (0 in ssy_on_dve)
        emit_tile0_b = None

        for t in range(NTILES):
            Xt = X[:, t, :]
            Yt = Y[:, t, :]
            Pt = PT[:, t, :]
            rn_t = ACC[:, RN0 + 2 * t:RN0 + 2 * t + 2]
            rnx = ACC[:, RN0 + 2 * t:RN0 + 2 * t + 1]
            rny = ACC[:, RN0 + 2 * t + 1:RN0 + 2 * t + 2]
            dt_ = ACC[:, DT0 + t:DT0 + t + 1]

            # ---- POOL: p_t = x * y (independent of rn; runs early)
            nc.gpsimd.wait_ge(dx[t], 16)
            nc.gpsimd.wait_ge(dy[t], 16)
            nc.gpsimd.tensor_mul(Pt, Xt, Yt).then_inc(sg, 1)
            sg_t += 1
            p_tick[t] = sg_t

            # ---- ACT: ssx via Square-with-accum
            nc.scalar.wait_ge(dx[t], 16)
            nc.scalar.activation(
                out=scra[:, 0, :], in_=Xt, func=Square, bias=zb,
                accum_out=ss[:, t, 0:1],
            ).then_inc(sa, 1)
            sa_t += 1
            sqx_tick = sa_t

            if t == 1 and emit_tile0_b is not None and sqrty0_after_sqx1:
                sa_t, sv_t = emit_tile0_b(sa_t, sv_t)
                emit_tile0_b = None

            if t == 0 and split_rn0:
                # ---- tile 0, split norm chain: xn0 only needs rnx
                nc.vector.wait_ge(dy[0], 16)
                nc.vector.scalar_tensor_tensor(
                    out=scrv, in0=Yt, scalar=1.0, in1=Yt,
                    op0=bypass, op1=mult, accum_out=ss[:, 0, 1:2],
                ).then_inc(sv, 1)
                sv_t += 1
                ssy0_tick = sv_t

                nc.scalar.wait_ge(sa, sqx_tick)
                nc.scalar.activation(
                    out=nrm[:, 0, 0:1], in_=ss[:, 0, 0:1], func=Sqrt, bias=zb,
                ).then_inc(sa, 1)
                sa_t += 1
                sqrtx0_tick = sa_t

                nc.vector.wait_ge(sa, sqrtx0_tick)
                nc.vector.reciprocal(out=rnx, in_=nrm[:, 0, 0:1]).then_inc(sv, 1)
                sv_t += 1
                recip_tick[0] = sv_t

                nc.vector.wait_ge(sv, recip_tick[0])
                nc.vector.tensor_scalar_mul(xnacc, Xt, rnx).then_inc(sv, 1)
                sv_t += 1
                xnacc_tick[0] = sv_t

                def emit_tile0_b(sa_n, sv_n, Yt=Yt, Pt=Pt, rny=rny, dt_=dt_,
                                 ssy0_tick=ssy0_tick, rnx=rnx):
                    nc.scalar.wait_ge(sv, ssy0_tick)
                    nc.scalar.activation(
                        out=nrm[:, 0, 1:2], in_=ss[:, 0, 1:2], func=Sqrt,
                        bias=zb,
                    ).then_inc(sa, 1)
                    sa_n += 1
                    nc.vector.wait_ge(sa, sa_n)
                    nc.vector.reciprocal(
                        out=rny, in_=nrm[:, 0, 1:2]
                    ).then_inc(sv, 1)
                    sv_n += 1
                    nc.vector.wait_ge(sv, sv_n)
                    nc.vector.tensor_scalar_mul(ynacc, Yt, rny).then_inc(sv, 1)
                    sv_n += 1
                    nc.vector.wait_ge(sg, p_tick[0])
                    nc.vector.tensor_scalar(
                        out=scrv, in0=Pt, scalar1=rnx, scalar2=0.0,
                        op0=mult, op1=add, accum_out=dt_,
                    ).then_inc(sv, 1)
                    sv_n += 1
                    return sa_n, sv_n

                if not sqrty0_after_sqx1:
                    sa_t, sv_t = emit_tile0_b(sa_t, sv_t)
                    emit_tile0_b = None
                continue

            if t in ssy_on_dve:
                # ---- DVE: ssy_t -- balances ACT/DVE load; also pulls the
                # tile-0 sqrt earlier than the serial ACT chain would allow
                nc.vector.wait_ge(dy[t], 16)
                nc.vector.scalar_tensor_tensor(
                    out=scrv, in0=Yt, scalar=1.0, in1=Yt,
                    op0=bypass, op1=mult, accum_out=ss[:, t, 1:2],
                ).then_inc(sv, 1)
                sv_t += 1
                ssy_sv_tick = sv_t
                # sqrt waits own sqx accum + DVE ssy
                nc.scalar.wait_ge(sa, sqx_tick)
                nc.scalar.wait_ge(sv, ssy_sv_tick)
            else:
                nc.scalar.wait_ge(dy[t], 16)
                nc.scalar.activation(
                    out=scra[:, 1, :], in_=Yt, func=Square, bias=zb,
                    accum_out=ss[:, t, 1:2],
                ).then_inc(sa, 1)
                sa_t += 1
                nc.scalar.wait_ge(sa, sa_t)  # own accum-out hazard

            # ---- ACT: nrm = sqrt(ss)
            nc.scalar.activation(
                out=nrm[:, t, :], in_=ss[:, t, :], func=Sqrt, bias=zb,
            ).then_inc(sa, 1)
            sa_t += 1
            sqrt_tick[t] = sa_t

            # ---- DVE: rn = 1/nrm
            nc.vector.wait_ge(sa, sqrt_tick[t])
            nc.vector.reciprocal(out=rn_t, in_=nrm[:, t, :]).then_inc(sv, 1)
            sv_t += 1
            recip_tick[t] = sv_t

            # ---- DVE: xnacc += rnx * x   (scalar-ptr setup hazard on rn)
            nc.vector.wait_ge(sv, recip_tick[t])
            if t == 0:
                nc.vector.tensor_scalar_mul(xnacc, Xt, rnx).then_inc(sv, 1)
            else:
                nc.vector.scalar_tensor_tensor(
                    out=xnacc, in0=Xt, scalar=rnx, in1=xnacc,
                    op0=mult, op1=add,
                ).then_inc(sv, 1)
            sv_t += 1
            xnacc_tick[t] = sv_t

            # ---- DVE: ynacc += rny * y
            if t == 0:
                nc.vector.tensor_scalar_mul(ynacc, Yt, rny).then_inc(sv, 1)
            else:
                nc.vector.scalar_tensor_tensor(
                    out=ynacc, in0=Yt, scalar=rny, in1=ynacc,
                    op0=mult, op1=add,
                ).then_inc(sv, 1)
            sv_t += 1

            # ---- dtmp_t = sum_d rnx * p_t
            if t in dg_on_act:
                # ACT is idle after its last sqrt: Copy-with-scale + accum
                # takes the diag off the DVE tail for the final tile(s)
                nc.scalar.wait_ge(sg, p_tick[t])
                nc.scalar.wait_ge(sv, recip_tick[t])
                nc.scalar.activation(
                    out=scra[:, 0, :], in_=Pt,
                    func=mybir.ActivationFunctionType.Copy,
                    scale=rnx, accum_out=dt_,
                ).then_inc(sa, 1)
                sa_t += 1
            else:
                # DVE single-stream tensor_scalar (2x mode)
                nc.vector.wait_ge(sg, p_tick[t])
                nc.vector.tensor_scalar(
                    out=scrv, in0=Pt, scalar1=rnx, scalar2=0.0,
                    op0=mult, op1=add, accum_out=dt_,
                ).then_inc(sv, 1)
                sv_t += 1

        # ---- tail: xnacc out on the ACT HWDGE ring as soon as it's final,
        # the rest on the SP ring once the last DVE op lands.
        nc.scalar.wait_ge(sv, xnacc_tick[NTILES - 1])
        nc.scalar.dma_start(out=o_d[:, 0:D], in_=xnacc).then_inc(do_, 16)
        nc.sync.wait_ge(sv, sv_t)
        if dg_on_act:
            nc.sync.wait_ge(sa, sa_t)
        nc.sync.dma_start(out=o_d[:, D:OUTW], in_=ACC[:, D:OUTW]).then_inc(do_, 16)
        if final_wait:
            nc.sync.wait_ge(do_, 32)

    return nc


def _get_program():
    if "nc" not in _CACHE:
        _CACHE["nc"] = _build_program()
    return _CACHE["nc"]


def kernel(x: np.ndarray, y: np.ndarray) -> np.ndarray:
    from concourse import bass_utils

    nc = _get_program()

    xs = np.ascontiguousarray(np.asarray(x, dtype=np.float32).reshape(N, D))
    ys = np.ascontiguousarray(np.asarray(y, dtype=np.float32).reshape(N, D))

    in_maps = [
        {"x": xs[c * ROWS:(c + 1) * ROWS], "y": ys[c * ROWS:(c + 1) * ROWS]}
        for c in range(NCORES)
    ]
    res = bass_utils.run_bass_kernel_spmd(nc, in_maps, core_ids=list(range(NCORES)))

    sx = np.zeros(D, dtype=np.float64)
    sy = np.zeros(D, dtype=np.float64)
    dg = 0.0
    for r in res.results:
        o = r["o"].astype(np.float64)
        sx += o[:, 0:D].sum(axis=0)
        sy += o[:, D:2 * D].sum(axis=0)
        dtmp = o[:, DT0:DT0 + NTILES]                    # rnx * (x . y)
        rny = o[:, RN0 + 1:RN0 + 2 * NTILES:2]           # (P, NTILES)
        dg += (dtmp * rny).sum()

    val = (np.dot(sx, sy) - dg) / float(N * N)
    return np.array(val, dtype=np.float32)


# revision 36
# speedup vs baseline: 1.0297x; 1.0143x over previous
"""Cosine-similarity loss kernel for Trainium2 (8 NeuronCores, SPMD).

Computes mean(offdiag(xn @ yn^T)) for row-normalized x, y of shape
(N, D) = (4096, 512), via the identity

    mean = [ (Sx . Sy) - sum_n (xn_n . yn_n) ] / N^2
    Sx = sum_n xn_n,  Sy = sum_m yn_m

so the N x N similarity matrix is never formed. Rows are sharded across
the 8 cores (512 rows each, 4 partition-tiles of 128). Per core:

  POOL: p_t = x * y elementwise (runs right after the DMAs, off the
        critical path)
  ACT : row sums-of-squares (Square activation with accum), sqrt
        (Square/Sqrt share one activation table: sqrt_and_friends)
  DVE : 1/norm, xnacc += rnx*x, ynacc += rny*y,
        dtmp_t = sum_d rnx*p_t (single-stream tensor_scalar, 2x mode)
  out : xnacc via the ACT HWDGE ring as soon as it is final;
        [ynacc | dtmp | rn] via the SP ring. Host sums partitions/cores,
        applies the rny factor to dtmp, and takes the 512-length dot.

Raw bass with explicit semaphores; every instruction carries at most one
sync-wait (this walrus rejects multi-wait instructions); cross-engine
ordering is established transitively through semaphore chains.
"""

import numpy as np

N = 4096
D = 512
NCORES = 8
ROWS = N // NCORES  # 512 rows per core
P = 128
NTILES = ROWS // P  # 4
# ACC columns: [xnacc 0:512 | ynacc 512:1024 | dtmp 1024:1028 | rn 1028:1036]
DT0 = 2 * D
RN0 = 2 * D + NTILES
OUTW = RN0 + 2 * NTILES

_CACHE = {}

# tiles whose ssy (row sum-of-squares of y) runs on DVE instead of ACT --
# balances the two engines' per-tile load (tuned via cost-model sweep)
SSY_ON_DVE = (0,)
FINAL_WAIT = True
# tiles whose diag reduction runs on ACT (Copy with scale AP + accum) --
# ACT is idle after the last sqrt, so this takes the diag off the DVE tail
DG_ON_ACT = (3,)
# split tile-0's sqrt/recip into separate x and y halves: xn0 only needs
# rnx, so it can start before ssy0 -> sqrt(y) resolves; ACT's bubble
# before the combined sqrt0 is filled with sqx1 instead
SPLIT_RN0 = True
# place tile-0's sqrt(y) after sqx1 in the ACT stream (True) or
# immediately after sqrt(x) (False)
SQRTY0_AFTER_SQX1 = True
# split the LAST tile's sqrt/recip the same way: recipx/xn can start while
# ACT is still squaring y, hiding the cross-engine hops on the tail
SPLIT_RN_LAST = True
# input DMA issue order (tuned via cost-model sweep)
DMA_ORDER = (("x", 0), ("y", 0), ("x", 1), ("y", 1),
             ("x", 2), ("y", 2), ("x", 3), ("y", 3))


def _build_program(ssy_on_dve=None, final_wait=None, dma_order=None,
                   dg_on_act=None, split_rn0=None, sqrty0_after_sqx1=None,
                   split_rn_last=None):
    import concourse.bass as bass
    from concourse import mybir
    from contextlib import ExitStack

    if ssy_on_dve is None:
        ssy_on_dve = SSY_ON_DVE
    if final_wait is None:
        final_wait = FINAL_WAIT
    if dma_order is None:
        dma_order = DMA_ORDER
    if dg_on_act is None:
        dg_on_act = DG_ON_ACT
    if split_rn0 is None:
        split_rn0 = SPLIT_RN0
    if sqrty0_after_sqx1 is None:
        sqrty0_after_sqx1 = SQRTY0_AFTER_SQX1
    if split_rn_last is None:
        split_rn_last = SPLIT_RN_LAST

    f32 = mybir.dt.float32
    mult = mybir.AluOpType.mult
    add = mybir.AluOpType.add
    bypass = mybir.AluOpType.bypass
    Square = mybir.ActivationFunctionType.Square
    Sqrt = mybir.ActivationFunctionType.Sqrt

    nc = bass.Bass("TRN2", target_bir_lowering=False)

    x_d = nc.dram_tensor("x", [ROWS, D], f32, kind="ExternalInput").ap()
    y_d = nc.dram_tensor("y", [ROWS, D], f32, kind="ExternalInput").ap()
    o_d = nc.dram_tensor("o", [P, OUTW], f32, kind="ExternalOutput").ap()

    with (
        ExitStack() as _sems,
        nc.semaphore("sv") as sv,    # DVE progress
        nc.semaphore("sa") as sa,    # ACT progress
        nc.semaphore("sg") as sg,    # POOL progress
        nc.semaphore("sz") as sz,    # zero-bias tile ready
        nc.semaphore("do_") as do_,  # output DMA completion
        nc.sbuf_tensor("zb", [P, 1], f32) as zb_h,
        nc.sbuf_tensor("X", [P, NTILES, D], f32) as X_h,
        nc.sbuf_tensor("Y", [P, NTILES, D], f32) as Y_h,
        nc.sbuf_tensor("PT", [P, NTILES, D], f32) as PT_h,
        nc.sbuf_tensor("ss", [P, NTILES, 2], f32) as ss_h,
        nc.sbuf_tensor("nrm", [P, NTILES, 2], f32) as nrm_h,
        nc.sbuf_tensor("scrv", [P, D], f32) as scrv_h,
        nc.sbuf_tensor("scra", [P, 2, D], f32) as scra_h,
        nc.sbuf_tensor("ACC", [P, OUTW], f32) as acc_h,
    ):
        X = X_h.ap()
        Y = Y_h.ap()
        PT = PT_h.ap()
        zb = zb_h.ap()
        ss = ss_h.ap()
        nrm = nrm_h.ap()
        scrv = scrv_h.ap()
        scra = scra_h.ap()
        ACC = acc_h.ap()
        xnacc = ACC[:, 0:D]
        ynacc = ACC[:, D:2 * D]

        # explicit zero bias for activations (avoids the implicit const-AP
        # path; framework preamble consts exist anyway, but zb keeps the
        # activation biases off the ACT critical path)
        nc.gpsimd.memset(zb, 0.0).then_inc(sz, 1)

        # ---- input DMAs: dedicated completion semaphore per DMA so every
        # wait is an unambiguous milestone (HWDGE completions can reorder).
        dx = [_sems.enter_context(nc.semaphore(f"dx{t}")) for t in range(NTILES)]
        dy = [_sems.enter_context(nc.semaphore(f"dy{t}")) for t in range(NTILES)]
        for kind, t in dma_order:
            if kind == "x":
                nc.sync.dma_start(
                    out=X[:, t, :], in_=x_d[t * P:(t + 1) * P, :]
                ).then_inc(dx[t], 16)
            else:
                nc.sync.dma_start(
                    out=Y[:, t, :], in_=y_d[t * P:(t + 1) * P, :]
                ).then_inc(dy[t], 16)

        nc.scalar.wait_ge(sz, 1)  # bias tile ready before first activation

        sa_t = 0  # ACT tick
        sv_t = 0  # DVE tick
        sg_t = 0  # POOL tick
        sqrt_tick = [0] * NTILES
        recip_tick = [0] * NTILES
        xnacc_tick = [0] * NTILES
        p_tick = [0] * NTILES

        split_rn0 = split_rn0 and (0 in ssy_on_dve)
        emit_tile0_b = None

        for t in range(NTILES):
            Xt = X[:, t, :]
            Yt = Y[:, t, :]
            Pt = PT[:, t, :]
            rn_t = ACC[:, RN0 + 2 * t:RN0 + 2 * t + 2]
            rnx = ACC[:, RN0 + 2 * t:RN0 + 2 * t + 1]
            rny = ACC[:, RN0 + 2 * t + 1:RN0 + 2 * t + 2]
            dt_ = ACC[:, DT0 + t:DT0 + t + 1]

            # ---- POOL: p_t = x * y (independent of rn; runs early)
            nc.gpsimd.wait_ge(dx[t], 16)
            nc.gpsimd.wait_ge(dy[t], 16)
            nc.gpsimd.tensor_mul(Pt, Xt, Yt).then_inc(sg, 1)
            sg_t += 1
            p_tick[t] = sg_t

            # ---- ACT: ssx via Square-with-accum
            nc.scalar.wait_ge(dx[t], 16)
            nc.scalar.activation(
                out=scra[:, 0, :], in_=Xt, func=Square, bias=zb,
                accum_out=ss[:, t, 0:1],
            ).then_inc(sa, 1)
            sa_t += 1
            sqx_tick = sa_t

            if t == 1 and emit_tile0_b is not None and sqrty0_after_sqx1:
                sa_t, sv_t = emit_tile0_b(sa_t, sv_t)
                emit_tile0_b = None

            if t == 0 and split_rn0:
                # ---- tile 0, split norm chain: xn0 only needs rnx
                nc.vector.wait_ge(dy[0], 16)
                nc.vector.scalar_tensor_tensor(
                    out=scrv, in0=Yt, scalar=1.0, in1=Yt,
                    op0=bypass, op1=mult, accum_out=ss[:, 0, 1:2],
                ).then_inc(sv, 1)
                sv_t += 1
                ssy0_tick = sv_t

                nc.scalar.wait_ge(sa, sqx_tick)
                nc.scalar.activation(
                    out=nrm[:, 0, 0:1], in_=ss[:, 0, 0:1], func=Sqrt, bias=zb,
                ).then_inc(sa, 1)
                sa_t += 1
                sqrtx0_tick = sa_t

                nc.vector.wait_ge(sa, sqrtx0_tick)
                nc.vector.reciprocal(out=rnx, in_=nrm[:, 0, 0:1]).then_inc(sv, 1)
                sv_t += 1
                recip_tick[0] = sv_t

                nc.vector.wait_ge(sv, recip_tick[0])
                nc.vector.tensor_scalar_mul(xnacc, Xt, rnx).then_inc(sv, 1)
                sv_t += 1
                xnacc_tick[0] = sv_t

                def emit_tile0_b(sa_n, sv_n, Yt=Yt, Pt=Pt, rny=rny, dt_=dt_,
                                 ssy0_tick=ssy0_tick, rnx=rnx):
                    nc.scalar.wait_ge(sv, ssy0_tick)
                    nc.scalar.activation(
                        out=nrm[:, 0, 1:2], in_=ss[:, 0, 1:2], func=Sqrt,
                        bias=zb,
                    ).then_inc(sa, 1)
                    sa_n += 1
                    nc.vector.wait_ge(sa, sa_n)
                    nc.vector.reciprocal(
                        out=rny, in_=nrm[:, 0, 1:2]
                    ).then_inc(sv, 1)
                    sv_n += 1
                    nc.vector.wait_ge(sv, sv_n)
                    nc.vector.tensor_scalar_mul(ynacc, Yt, rny).then_inc(sv, 1)
                    sv_n += 1
                    nc.vector.wait_ge(sg, p_tick[0])
                    nc.vector.tensor_scalar(
                        out=scrv, in0=Pt, scalar1=rnx, scalar2=0.0,
                        op0=mult, op1=add, accum_out=dt_,
                    ).then_inc(sv, 1)
                    sv_n += 1
                    return sa_n, sv_n

                if not sqrty0_after_sqx1:
                    sa_t, sv_t = emit_tile0_b(sa_t, sv_t)
                    emit_tile0_b = None
                continue

            if t == NTILES - 1 and split_rn_last and t not in ssy_on_dve:
                # ---- last tile, split norm chain: recipx/xn start while
                # ACT is still squaring y, hiding the tail's engine hops
                nc.scalar.wait_ge(sa, sqx_tick)
                nc.scalar.activation(
                    out=nrm[:, t, 0:1], in_=ss[:, t, 0:1], func=Sqrt, bias=zb,
                ).then_inc(sa, 1)
                sa_t += 1
                sqrtx_tick = sa_t

                nc.scalar.wait_ge(dy[t], 16)
                nc.scalar.activation(
                    out=scra[:, 1, :], in_=Yt, func=Square, bias=zb,
                    accum_out=ss[:, t, 1:2],
                ).then_inc(sa, 1)
                sa_t += 1
                nc.scalar.wait_ge(sa, sa_t)  # own accum-out hazard
                nc.scalar.activation(
                    out=nrm[:, t, 1:2], in_=ss[:, t, 1:2], func=Sqrt, bias=zb,
                ).then_inc(sa, 1)
                sa_t += 1
                sqrty_tick = sa_t

                nc.vector.wait_ge(sa, sqrtx_tick)
                nc.vector.reciprocal(out=rnx, in_=nrm[:, t, 0:1]).then_inc(sv, 1)
                sv_t += 1
                recip_tick[t] = sv_t

                nc.vector.wait_ge(sv, recip_tick[t])
                nc.vector.scalar_tensor_tensor(
                    out=xnacc, in0=Xt, scalar=rnx, in1=xnacc,
                    op0=mult, op1=add,
                ).then_inc(sv, 1)
                sv_t += 1
                xnacc_tick[t] = sv_t

                nc.vector.wait_ge(sa, sqrty_tick)
                nc.vector.reciprocal(out=rny, in_=nrm[:, t, 1:2]).then_inc(sv, 1)
                sv_t += 1
                recipy_tick = sv_t

                nc.vector.wait_ge(sv, recipy_tick)
                nc.vector.scalar_tensor_tensor(
                    out=ynacc, in0=Yt, scalar=rny, in1=ynacc,
                    op0=mult, op1=add,
                ).then_inc(sv, 1)
                sv_t += 1
            else:
                if t in ssy_on_dve:
                    # ---- DVE: ssy_t -- balances ACT/DVE load; also pulls the
                    # tile-0 sqrt earlier than the serial ACT chain would allow
                    nc.vector.wait_ge(dy[t], 16)
                    nc.vector.scalar_tensor_tensor(
                        out=scrv, in0=Yt, scalar=1.0, in1=Yt,
                        op0=bypass, op1=mult, accum_out=ss[:, t, 1:2],
                    ).then_inc(sv, 1)
                    sv_t += 1
                    ssy_sv_tick = sv_t
                    # sqrt waits own sqx accum + DVE ssy
                    nc.scalar.wait_ge(sa, sqx_tick)
                    nc.scalar.wait_ge(sv, ssy_sv_tick)
                else:
                    nc.scalar.wait_ge(dy[t], 16)
                    nc.scalar.activation(
                        out=scra[:, 1, :], in_=Yt, func=Square, bias=zb,
                        accum_out=ss[:, t, 1:2],
                    ).then_inc(sa, 1)
                    sa_t += 1
                    nc.scalar.wait_ge(sa, sa_t)  # own accum-out hazard

                # ---- ACT: nrm = sqrt(ss)
                nc.scalar.activation(
                    out=nrm[:, t, :], in_=ss[:, t, :], func=Sqrt, bias=zb,
                ).then_inc(sa, 1)
                sa_t += 1
                sqrt_tick[t] = sa_t

                # ---- DVE: rn = 1/nrm
                nc.vector.wait_ge(sa, sqrt_tick[t])
                nc.vector.reciprocal(out=rn_t, in_=nrm[:, t, :]).then_inc(sv, 1)
                sv_t += 1
                recip_tick[t] = sv_t

                # ---- DVE: xnacc += rnx * x  (scalar-ptr setup hazard on rn)
                nc.vector.wait_ge(sv, recip_tick[t])
                if t == 0:
                    nc.vector.tensor_scalar_mul(xnacc, Xt, rnx).then_inc(sv, 1)
                else:
                    nc.vector.scalar_tensor_tensor(
                        out=xnacc, in0=Xt, scalar=rnx, in1=xnacc,
                        op0=mult, op1=add,
                    ).then_inc(sv, 1)
                sv_t += 1
                xnacc_tick[t] = sv_t

                # ---- DVE: ynacc += rny * y
                if t == 0:
                    nc.vector.tensor_scalar_mul(ynacc, Yt, rny).then_inc(sv, 1)
                else:
                    nc.vector.scalar_tensor_tensor(
                        out=ynacc, in0=Yt, scalar=rny, in1=ynacc,
                        op0=mult, op1=add,
                    ).then_inc(sv, 1)
                sv_t += 1

            # ---- dtmp_t = sum_d rnx * p_t
            if t in dg_on_act:
                # ACT is idle after its last sqrt: Copy-with-scale + accum
                # takes the diag off the DVE tail for the final tile(s)
                nc.scalar.wait_ge(sg, p_tick[t])
                nc.scalar.wait_ge(sv, recip_tick[t])
                nc.scalar.activation(
                    out=scra[:, 0, :], in_=Pt,
                    func=mybir.ActivationFunctionType.Copy,
                    scale=rnx, accum_out=dt_,
                ).then_inc(sa, 1)
                sa_t += 1
            else:
                # DVE single-stream tensor_scalar (2x mode)
                nc.vector.wait_ge(sg, p_tick[t])
                nc.vector.tensor_scalar(
                    out=scrv, in0=Pt, scalar1=rnx, scalar2=0.0,
                    op0=mult, op1=add, accum_out=dt_,
                ).then_inc(sv, 1)
                sv_t += 1

        # ---- tail: xnacc out on the ACT HWDGE ring as soon as it's final,
        # the rest on the SP ring once the last DVE op lands.
        nc.scalar.wait_ge(sv, xnacc_tick[NTILES - 1])
        nc.scalar.dma_start(out=o_d[:, 0:D], in_=xnacc).then_inc(do_, 16)
        nc.sync.wait_ge(sv, sv_t)
        if dg_on_act:
            nc.sync.wait_ge(sa, sa_t)
        nc.sync.dma_start(out=o_d[:, D:OUTW], in_=ACC[:, D:OUTW]).then_inc(do_, 16)
        if final_wait:
            nc.sync.wait_ge(do_, 32)

    return nc


def _get_program():
    if "nc" not in _CACHE:
        _CACHE["nc"] = _build_program()
    return _CACHE["nc"]


def kernel(x: np.ndarray, y: np.ndarray) -> np.ndarray:
    from concourse import bass_utils

    nc = _get_program()

    xs = np.ascontiguousarray(np.asarray(x, dtype=np.float32).reshape(N, D))
    ys = np.ascontiguousarray(np.asarray(y, dtype=np.float32).reshape(N, D))

    in_maps = [
        {"x": xs[c * ROWS:(c + 1) * ROWS], "y": ys[c * ROWS:(c + 1) * ROWS]}
        for c in range(NCORES)
    ]
    res = bass_utils.run_bass_kernel_spmd(nc, in_maps, core_ids=list(range(NCORES)))

    sx = np.zeros(D, dtype=np.float64)
    sy = np.zeros(D, dtype=np.float64)
    dg = 0.0
    for r in res.results:
        o = r["o"].astype(np.float64)
        sx += o[:, 0:D].sum(axis=0)
        sy += o[:, D:2 * D].sum(axis=0)
        dtmp = o[:, DT0:DT0 + NTILES]                    # rnx * (x . y)
        rny = o[:, RN0 + 1:RN0 + 2 * NTILES:2]           # (P, NTILES)
        dg += (dtmp * rny).sum()

    val = (np.dot(sx, sy) - dg) / float(N * N)
    return np.array(val, dtype=np.float32)
